# revision 133
# baseline (speedup 1.0000x reference)
"""Causal self-attention (B=8, T=1024, C=768, NH=12) on 8 TRN2 NeuronCores.

Sharding: pure data parallel - one batch element per core, no collectives.

Host side: x, w_attn, w_proj are pre-cast to bf16 (numpy/ml_dtypes) and fed
to the device program as bf16 DRAM tensors; biases stay fp32.

Per-core kernel (Bass/Tile), all intermediates resident in SBUF:
  1. xT = x.T (bf16) via PE transposes               [C=768, T=1024]
  2. qkT = w_qk-stationary matmul -> [2C, T] (q/k heads land pre-transposed
     [HD, T]); v kept row-major [T, C] with an appended ones column per head
     (v_aug [T, 12*65]) so the attention row-sums fall out of the PV matmul.
  3. Per head pair (2 heads share a 128-partition tile), per 512-wide q block:
       ST = kT.T @ qT -> PSUM [tk=128, tq=512]  (2 heads in PE row groups)
       U  = exp(0.125 * ST) (ACT; scores are bounded, exp is safe), bf16 SBUF
       diagonal-crossing 128-col blocks get a multiplicative triangular mask
       yq[q=128, 65] += U[:, qt-block].T-as-stationary @ v_aug[tk, head]
         (U is the STATIONARY operand, v_aug [128, 65] the moving one: the
          matmul costs 65 streamed columns instead of 512, and the y output
          lands q-major so softmax normalization is a per-partition scalar)
       normalize: recip(rowsum col) on DVE, broadcast along free dim.
  4. yT = y.T via PE transposes; out = yT.T @ w_proj + b_proj -> [T, C] fp32.

Matmul operands are bf16 (PE full rate); PSUM accumulation is fp32. SBUF
pools are never recycled for DMA-written tiles (address reuse makes later
tiles inherit sync deps on DMA queues, overflowing walrus's sync budgets);
PSUM / compute-only SBUF pools do recycle.
"""

import numpy as np
import ml_dtypes

import concourse.bass as bass
import concourse.bacc as bacc
import concourse.tile as tile
from concourse import mybir
from concourse.bass_utils import run_bass_kernel_spmd

B, T, C = 8, 1024, 768
NH, HD = 12, 64
P = 128
KC = C // P          # 6 k-tiles over C
KT = T // P          # 8 tiles over T
NQK = 2 * C // P     # 12 m-tiles for q+k
NHP = NH // 2        # 6 head pairs
TQB = 512            # tq block (one PSUM bank of fp32)
NB = T // TQB        # 2 tq blocks
NQT = TQB // P       # 4 q sub-tiles of 128 per block
VW = HD + 1          # 65: v columns + ones column per head

F32 = mybir.dt.float32
BF16 = mybir.dt.bfloat16
FP8 = mybir.dt.float8e4
FT = mybir.ActivationFunctionType


def build_program():
    nc = bacc.Bacc("TRN2", target_bir_lowering=False, debug=False)
    xb_d = nc.dram_tensor("xb", [T, C], BF16, kind="ExternalInput").ap()
    wab_d = nc.dram_tensor("wab", [C, 3 * C], BF16, kind="ExternalInput").ap()
    wab8_d = nc.dram_tensor("wab8", [C, 2 * C], FP8, kind="ExternalInput").ap()
    ba_d = nc.dram_tensor("b_attn", [3 * C], F32, kind="ExternalInput").ap()
    wpb_d = nc.dram_tensor("wpb", [C, C], BF16, kind="ExternalInput").ap()
    bp_d = nc.dram_tensor("b_proj", [C], F32, kind="ExternalInput").ap()
    out_d = nc.dram_tensor("out", [T, C], F32, kind="ExternalOutput").ap()

    from contextlib import ExitStack

    with tile.TileContext(nc) as tc:
        with ExitStack() as ctx:
            _body(ctx, tc, xb_d, wab_d, wab8_d, ba_d, wpb_d, bp_d, out_d)
    nc.compile()
    return nc


def _body(ctx, tc, xb_d, wab_d, wab8_d, ba_d, wpb_d, bp_d, out_d):
    nc = tc.nc

    const = ctx.enter_context(tc.tile_pool(name="const", bufs=1))
    persist = ctx.enter_context(tc.tile_pool(name="persist", bufs=1))
    wqk_pool = ctx.enter_context(tc.tile_pool(name="wqk", bufs=12))
    upool = ctx.enter_context(tc.tile_pool(name="upool", bufs=8))
    snorm = ctx.enter_context(tc.tile_pool(name="snorm", bufs=8))

    # constants ------------------------------------------------------------
    ident = const.tile([P, P], BF16)
    nc.gpsimd.memset(ident, 0.0)
    nc.gpsimd.affine_select(
        out=ident, in_=ident, compare_op=mybir.AluOpType.not_equal,
        fill=1.0, base=0, pattern=[[-1, P]], channel_multiplier=1,
    )
    # multiplicative causal mask: 1 where tk <= tq else 0 (applied post-exp)
    tri01 = const.tile([P, P], BF16)
    nc.gpsimd.memset(tri01, 1.0)
    nc.gpsimd.affine_select(
        out=tri01, in_=tri01, compare_op=mybir.AluOpType.is_ge,
        fill=0.0, base=0, pattern=[[1, P]], channel_multiplier=-1,
    )
    # persistent SBUF tensors ---------------------------------------------
    qkT = persist.tile([P, NQK, T], BF16)        # [128, 12, 1024]  3 MB
    vaug = persist.tile([P, KT, NH * VW], BF16)  # [128, 8, 780]  1.5 MB
    xT = persist.tile([P, KC, T], BF16)          # [128, 6, 1024] 1.5 MB
    xT8 = persist.tile([P, KC, T], FP8)          # fp8 x.T for the qk DR path
    wv_sb = persist.tile([P, KC, C], BF16)       # [128, 6, 768]
    wp_sb = persist.tile([P, KC, C], BF16)       # [128, 6, 768]
    # y in q-major layout: [q-part, m(8), head(12), 64] bf16
    ynorm = persist.tile([P, KT, NH, HD], BF16)  # 1.5 MB
    yTt = persist.tile([P, KC, T], BF16)         # y transposed  1.5 MB
    ot = persist.tile([P, KT, C], F32)           # [128, 8, 768]   3 MB

    # b_attn for q/k as per-partition scalars: [p, m] with b[128m + p]
    battn_pm = const.tile([P, NQK], F32)
    # b_attn v-part / b_proj broadcast along partitions: [128, 768]
    def _pbcast(src):
        return bass.AP(tensor=src.tensor, offset=src.offset, ap=[[0, P]] + list(src.ap))

    bv_b = const.tile([P, C], F32)
    bp_b = const.tile([P, C], F32)
    wt_tiles = {}

    def wt_load(m, eng):
        wt = wqk_pool.tile([P, KC, P], FP8, name=f"wt{m}", tag="wt")
        eng.dma_start(
            out=wt,
            in_=wab8_d[:, m * P : (m + 1) * P].rearrange("(k p) n -> p k n", p=P),
        )
        wt_tiles[m] = wt

    xbr = xb_d.rearrange("(t p) c -> p t c", p=P)
    x_all = persist.tile([P, KT, C], BF16, name="x_all")
    for q in range(KT):
        # alternate the issuing sequencer (SP / ACT are both HWDGE-capable):
        # two parallel issue streams halve the serial dma_start latency
        eng = nc.sync if q % 2 == 0 else nc.scalar
        eng.dma_start(out=x_all[:, q, :], in_=xbr[:, q, :])
    # first head pair's qk weights right after x, before the v weights
    wt_load(0, nc.sync)
    wt_load(6, nc.scalar)
    nc.sync.dma_start(
        out=battn_pm, in_=ba_d[0 : 2 * C].rearrange("(m p) -> p m", p=P)
    )
    for k in range(KC):
        nc.sync.dma_start(
            out=wv_sb[:, k, :], in_=wab_d[k * P : (k + 1) * P, 2 * C : 3 * C]
        )
    # the two 393KB broadcast bias DMAs go AFTER everything startup-critical:
    # bv_b is first read at ~15us (v drains), bp_b at ~90us (projection)
    nc.sync.dma_start(out=bv_b, in_=_pbcast(ba_d[2 * C : 3 * C]))
    nc.sync.dma_start(out=bp_b, in_=_pbcast(bp_d))

    # ---- interleaved phases 1+2 -----------------------------------------
    # v first; then per head pair: its two qk column-tiles followed by its
    # attention, so ACT/DVE overlap the next pair's QKV matmuls.
    # PSUM budget (8 banks): mm 2 (2x [128,512]) + spsum 4 (2x [128,2,512])
    # + ypsum 2 (2 head slots x [128,4,128]).
    with (
        tc.tile_pool(name="mmpsum", bufs=2, space="PSUM") as mmpsum,
        tc.tile_pool(name="spsum", bufs=2, space="PSUM") as spsum,
        tc.tile_pool(name="ypsum", bufs=1, space="PSUM") as ypsum,
    ):
        def qk_tile(m):
            if m in wt_tiles:
                wt = wt_tiles[m]
            else:
                wt = wqk_pool.tile([P, KC, P], FP8, name=f"wt{m}", tag="wt")
                nc.sync.dma_start(
                    out=wt,
                    in_=wab8_d[:, m * P : (m + 1) * P].rearrange(
                        "(k p) n -> p k n", p=P
                    ),
                )
            for n in range(NB):
                ps = mmpsum.tile([P, TQB], F32, name=f"qkps{m}{n}", tag="mm")
                for pr in range(KC // 2):
                    # fp8 DoubleRow: two 128-deep c-tiles per instruction at
                    # 0.5 cycles per streamed column
                    nc.tensor.matmul(
                        ps[:],
                        wt[:, 2 * pr : 2 * pr + 2, :],
                        xT8[:, 2 * pr : 2 * pr + 2, n * TQB : (n + 1) * TQB],
                        start=(pr == 0),
                        stop=(pr == KC // 2 - 1),
                        perf_mode=mybir.MatmulPerfMode.DoubleRow,
                    )
                nc.vector.tensor_tensor(
                    out=qkT[:, m, n * TQB : (n + 1) * TQB],
                    in0=ps[:],
                    in1=battn_pm[:, m : m + 1].to_broadcast([P, TQB]),
                    op=mybir.AluOpType.add,
                )

        # ---- phase 0: transpose x into xT (bf16 + fp8 copies) -----------
        # psums borrow the mm/pst slots (idle before the matmuls), so there
        # is no extra pool and no pool-close barrier after the transposes
        for t4 in range(KT // 4):
            for ck in range(KC):
                idx = t4 * KC + ck
                pool_, tg = (mmpsum, "mm") if idx % 2 == 0 else (spsum, "pst")
                pt = pool_.tile([P, 4, P], BF16, name=f"pt{t4}{ck}", tag=tg)
                for i in range(4):
                    nc.tensor.transpose(
                        pt[:, i, :],
                        x_all[:, t4 * 4 + i, ck * P : (ck + 1) * P],
                        ident[:],
                    )
                nc.vector.tensor_copy(
                    out=xT[:, ck, t4 * 4 * P : (t4 + 1) * 4 * P], in_=pt[:]
                )
                # fp8 copy of the same transposed block for the qk DR path
                nc.scalar.activation(
                    out=xT8[:, ck, t4 * 4 * P : (t4 + 1) * 4 * P],
                    in_=pt[:],
                    func=FT.Copy,
                )

        qk_tile(0)
        qk_tile(6)
        # ---- v rows (+bias), with interleaved ones cols ------------------
        vhe = vaug[:, :, :].rearrange("p t (h e) -> p t h e", e=VW)
        nc.vector.memset(vhe[:, :, :, HD : HD + 1], 1.0)
        def v_tile(tt):
            for n in range(NB):
                nsz = min(TQB, C - n * TQB)  # 512, 256
                # pre-attention v tiles use the idle scores slot: deeper
                # combined rotation against the DVE drains
                pool_, tg = (spsum, "pst") if tt < 4 and n == 0 else (mmpsum, "mm")
                ps = pool_.tile([P, TQB], F32, name=f"vps{tt}{n}", tag=tg)
                for k in range(KC):
                    nc.tensor.matmul(
                        ps[:, :nsz],
                        xT[:, k, tt * P : (tt + 1) * P],
                        wv_sb[:, k, n * TQB : n * TQB + nsz],
                        start=(k == 0),
                        stop=(k == KC - 1),
                    )
                nh0 = n * TQB // HD
                nh = nsz // HD
                nc.vector.tensor_tensor(
                    out=vhe[:, tt, nh0 : nh0 + nh, 0:HD],
                    in0=ps[:, :nsz].rearrange("p (h e) -> p h e", e=HD),
                    in1=bv_b[:, n * TQB : n * TQB + nsz].rearrange(
                        "p (h e) -> p h e", e=HD
                    ),
                    op=mybir.AluOpType.add,
                )

        for tt in range(4):
            v_tile(tt)

        # PV queue carried ACROSS q-block and head-pair boundaries: the exp
        # stream never waits for a block's PV flush + normalization; those
        # trail ~PV_DEPTH tiles behind inside the next block's score stream.
        pvq = []

        def flush_pv(tk, off, ut, hp, b, ntk, pys):
            # accumulation group spans the whole bank (zero region):
            # start on the first matmul into the bank, stop on the
            # last; the bank-wide pending-zero covers every qt slice.
            qt_min = max(0, tk - b * NQT)
            for h in range(2):
                for qt in range(qt_min, NQT):
                    nc.tensor.matmul(
                        pys[h][:, qt, 0:VW],
                        ut[:, h, qt * P : (qt + 1) * P],
                        vaug[:, tk, (2 * hp + h) * VW : (2 * hp + h + 1) * VW],
                        start=(tk == 0 and qt == 0),
                        stop=(tk == ntk - 1 and qt == NQT - 1),
                    )
            if tk == ntk - 1:
                # block finished: normalization (reciprocal of the rowsum
                # column, then multiply with free-dim broadcast)
                srec = snorm.tile([P, 2, NQT, 1], F32, name="srec")
                for h in range(2):
                    nc.vector.reciprocal(
                        out=srec[:, h, :, :], in_=pys[h][:, :, HD : HD + 1]
                    )
                for h in range(2):
                    nc.vector.tensor_tensor(
                        out=ynorm[:, b * NQT : (b + 1) * NQT, 2 * hp + h, :],
                        in0=pys[h][:, :, 0:HD],
                        in1=srec[:, h, :, :].to_broadcast([P, NQT, HD]),
                        op=mybir.AluOpType.mult,
                    )

        for hp in range(NHP):
            if hp > 0:
                qk_tile(hp)
                qk_tile(6 + hp)
            for b in range(NB):
                if hp == 0 and b == 1:
                    # second half of v overlapped with attention(0) block 0,
                    # so the exp stream starts ~10us earlier
                    for tt in range(4, KT):
                        v_tile(tt)
                ntk = 4 * (b + 1)
                # y accumulators: per head one PSUM bank [128, 4, 128],
                # matmuls write [:, qt, 0:65] (col 64 = rowsum).
                pys = [
                    ypsum.tile([P, NQT, P], F32, name=f"py{h}") for h in range(2)
                ]
                for tk in range(ntk):
                    diag = (tk // NQT) == b
                    off = tk * P - b * TQB if diag else 0
                    pst = spsum.tile([P, 2, TQB], F32, name="pst", tag="pst")
                    ut = upool.tile([P, 2, TQB], BF16, name="ut")
                    for h in range(2):
                        lo, hi = 64 * h, 64 * h + 64
                        nc.tensor.matmul(
                            pst[:, h, off:TQB],
                            qkT[lo:hi, 6 + hp, tk * P : (tk + 1) * P],
                            qkT[lo:hi, hp, b * TQB + off : (b + 1) * TQB],
                            start=True,
                            stop=True,
                        )
                    nc.scalar.activation(
                        out=ut[:, :, off:TQB],
                        in_=pst[:, :, off:TQB],
                        func=FT.Exp,
                        scale=0.125,
                    )
                    if diag:
                        nc.vector.tensor_tensor(
                            out=ut[:, :, off : off + P],
                            in0=ut[:, :, off : off + P],
                            in1=tri01[:, None, :].to_broadcast([P, 2, P]),
                            op=mybir.AluOpType.mult,
                        )
                    depth = 3 if (hp == NHP - 1 and b == NB - 1) else 6
                    while len(pvq) >= depth:
                        flush_pv(*pvq.pop(0))
                    pvq.append((tk, off, ut, hp, b, ntk, pys))
        for e in pvq:
            flush_pv(*e)

        # ---- phase 3: yT = y.T; out = yT.T @ w_proj + b_proj ------------
        # inside the same pool scope: no pool-close barrier before it; the
        # transpose / proj psums reuse the mm and pst slots.
        for k in range(KC):
            nc.sync.dma_start(
                out=wp_sb[:, k, :], in_=wpb_d[k * P : (k + 1) * P, :]
            )
        for m4 in range(KT // 4):
            for k in range(KC):
                pt = mmpsum.tile([P, 4, P], BF16, name=f"yt{m4}{k}", tag="mm")
                for i in range(4):
                    nc.tensor.transpose(
                        pt[:, i, :],
                        ynorm[:, m4 * 4 + i, 2 * k : 2 * k + 2, :],
                        ident[:],
                    )
                nc.vector.tensor_copy(
                    out=yTt[:, k, m4 * 4 * P : (m4 + 1) * 4 * P], in_=pt[:]
                )
        for m in range(KT):
            for n in range(NB):
                nsz = min(TQB, C - n * TQB)
                pool_, tg = (spsum, "pst") if n == 0 else (mmpsum, "mm")
                ps = pool_.tile([P, TQB], F32, name=f"ops{m}{n}", tag=tg)
                for k in range(KC):
                    nc.tensor.matmul(
                        ps[:, :nsz],
                        yTt[:, k, m * P : (m + 1) * P],
                        wp_sb[:, k, n * TQB : n * TQB + nsz],
                        start=(k == 0),
                        stop=(k == KC - 1),
                    )
                nc.vector.tensor_tensor(
                    out=ot[:, m, n * TQB : n * TQB + nsz],
                    in0=ps[:, :nsz],
                    in1=bp_b[:, n * TQB : n * TQB + nsz],
                    op=mybir.AluOpType.add,
                )
            for n in range(NB):
                nsz = min(TQB, C - n * TQB)
                nc.sync.dma_start(
                    out=out_d.rearrange("(t p) c -> p t c", p=P)[
                        :, m : m + 1, n * TQB : n * TQB + nsz
                    ],
                    in_=ot[:, m : m + 1, n * TQB : n * TQB + nsz],
                )


_prog_cache = {}


def _get_program():
    if "nc" not in _prog_cache:
        _prog_cache["nc"] = build_program()
    return _prog_cache["nc"]


def kernel(x, w_attn, b_attn, w_proj, b_proj, _trace=False):
    nc = _get_program()
    bf = ml_dtypes.bfloat16
    f8 = ml_dtypes.float8_e4m3
    xf = np.asarray(x, dtype=np.float32)
    wf = np.asarray(w_attn, dtype=np.float32)
    xb = np.ascontiguousarray(xf.astype(bf))
    wab = np.ascontiguousarray(wf.astype(bf))
    wab8 = np.ascontiguousarray(wf[:, 0 : 2 * C].astype(f8))
    wpb = np.ascontiguousarray(np.asarray(w_proj, dtype=np.float32).astype(bf))
    b_attn = np.ascontiguousarray(np.asarray(b_attn, dtype=np.float32))
    b_proj = np.ascontiguousarray(np.asarray(b_proj, dtype=np.float32))
    in_maps = [
        {
            "xb": xb[b],
            "wab": wab,
            "wab8": wab8,
            "b_attn": b_attn,
            "wpb": wpb,
            "b_proj": b_proj,
        }
        for b in range(B)
    ]
    res = run_bass_kernel_spmd(nc, in_maps, list(range(B)), trace=_trace)
    out = np.stack([res.results[i]["out"] for i in range(B)], axis=0)
    if _trace:
        kernel.last_results = res
    return out
